# revision 31
# baseline (speedup 1.0000x reference)
"""Multi-head attention (dense transformer block) on 8 Trainium2 NeuronCores.

Reference computation (per batch element b of 8):
    qkv = x @ w_qkv.T + b_qkv                  # [1024, 2304]
    q, k, v = split heads (12 heads, d=64)
    attn = softmax(q k^T / sqrt(d))
    out  = (attn v) reshaped @ w_proj.T + b_proj

Sharding: pure data parallel — core b handles batch element b, weights are
replicated, no collectives.

Per-core kernel (all matmul operands fp16, fp32 PSUM accumulation):
  C: v    = x Wv^T + b_v            -> [1024, 12*(64+1)] (ones col per head
     makes the PV matmul emit softmax row-sums for free)
  B: qk^T = [Wq*scale; Wk] x^T      -> [1536, 1024] (features on partitions)
  D: per head pair hp: scores^T = k^T q (two heads row-tiled into the two
     halves of the PE array), exp on ScalarE straight from PSUM pairs,
     PV accumulate, normalize by approx-reciprocal(rowsum).
     B tiles for wave hp+1 and PV of wave hp-1 are interleaved between the
     score matmuls so the PE never idles while ScalarE runs the exps.
  E: out = score w_proj^T + b_proj  (bias via ones-row matmul)
"""

import os
import sys

for _p in ("/opt/trn_rl_repo", "/root/.axon_site/_ro/trn_rl_repo"):
    if os.path.isdir(_p) and _p not in sys.path:
        sys.path.insert(0, _p)

import numpy as np

import concourse.bass as bass
import concourse.mybir as mybir
import concourse.tile as tile
from concourse import bacc
from concourse.bass_utils import run_bass_kernel_spmd

DIM = 768
N_HEAD = 12
HEAD_DIM = 64
SCALE = HEAD_DIM ** (-0.5)
NB = 8          # batch == number of cores
N = 1024        # sequence length
CCH = DIM // 128  # 6 contraction chunks

F32 = mybir.dt.float32
F16 = mybir.dt.float16
AF = mybir.ActivationFunctionType

_CACHE: dict = {}


def _build():
    nc = bacc.Bacc("TRN2", target_bir_lowering=False, debug=False)

    xT_d = nc.dram_tensor("xT", [DIM, N], F16, kind="ExternalInput")
    wqk_d = nc.dram_tensor("w_qk_T", [DIM, 2 * DIM], F16, kind="ExternalInput")
    bqk_d = nc.dram_tensor("b_qk_t", [128, 12], F32, kind="ExternalInput")
    wv_d = nc.dram_tensor("w_v_T", [DIM, DIM], F16, kind="ExternalInput")
    bv_d = nc.dram_tensor("b_v", [1, DIM], F16, kind="ExternalInput")
    wp_d = nc.dram_tensor("w_p_T", [DIM, DIM], F16, kind="ExternalInput")
    bp_d = nc.dram_tensor("b_p", [1, DIM], F16, kind="ExternalInput")
    out_d = nc.dram_tensor("out", [N, DIM], F32, kind="ExternalOutput")

    with tile.TileContext(nc) as tc:
        with (
            tc.tile_pool(name="consts", bufs=1) as consts,
            tc.tile_pool(name="qk", bufs=1) as qk_pool,
            tc.tile_pool(name="score", bufs=1) as score_pool,
            tc.tile_pool(name="v", bufs=1) as v_pool,
            tc.tile_pool(name="x", bufs=1) as x_pool,
            tc.tile_pool(name="wqk", bufs=1) as wqk_pool,
            tc.tile_pool(name="wv", bufs=1) as wv_pool,
            tc.tile_pool(name="wp", bufs=1) as wp_pool,
            tc.tile_pool(name="attn", bufs=32) as attn_pool,
            tc.tile_pool(name="small", bufs=4) as small_pool,
            tc.tile_pool(name="ostage", bufs=2) as out_pool,
            tc.tile_pool(name="ps", bufs=2, space="PSUM") as ps_pool,
            tc.tile_pool(name="pair", bufs=2, space="PSUM") as pair_pool,
            tc.tile_pool(name="acc", bufs=2, space="PSUM") as acc_pool,
        ):
            x_sb = x_pool.tile([128, CCH, N], F16)
            wv_sb = wv_pool.tile([128, CCH, DIM], F16)
            wqk_sb = wqk_pool.tile([128, CCH, 2 * DIM], F16)
            wp_sb = wp_pool.tile([128, CCH, DIM], F16)

            ones_sb = consts.tile([1, 128], F16)
            nc.vector.memset(ones_sb[:], 1.0)
            bqk_sb = consts.tile([128, 12], F32)
            nc.sync.dma_start(bqk_sb[:], bqk_d[:])
            bv_sb = consts.tile([1, DIM], F16)
            nc.sync.dma_start(bv_sb[:], bv_d[:])
            bp_sb = consts.tile([1, DIM], F16)
            nc.sync.dma_start(bp_sb[:], bp_d[:])

            # phase C consumes x in n-order and wv in o-order: land the
            # first halves of each (split across both DGE queues) before the
            # rest so its matmuls start early
            xT_v = xT_d[:].rearrange("(c p) n -> p c n", p=128)
            wv_v = wv_d[:].rearrange("(c p) o -> p c o", p=128)
            nc.sync.dma_start(wv_sb[:, :, 0:512], wv_v[:, :, 0:512])
            nc.sync.dma_start(x_sb[:, :, 0:512], xT_v[:, :, 0:512])
            nc.sync.dma_start(wv_sb[:, :, 512:DIM], wv_v[:, :, 512:DIM])
            nc.sync.dma_start(x_sb[:, :, 512:N], xT_v[:, :, 512:N])
            wqk_v = wqk_d[:].rearrange("(c p) o -> c p o", p=128)
            for c in range(CCH):
                nc.sync.dma_start(wqk_sb[:, c, :], wqk_v[c])
            nc.sync.dma_start(wp_sb[:], wp_d[:].rearrange("(c p) o -> p c o", p=128))

            # broadcast b_v / b_p across partitions once; the evictions add
            # them on the DVE, saving 32 ones-row bias matmuls on the PE
            bv32 = consts.tile([1, DIM], F32)
            nc.vector.tensor_copy(bv32[:], bv_sb[:])
            bvb = consts.tile([128, DIM], F32)
            nc.gpsimd.partition_broadcast(bvb[:], bv32[:], channels=128)
            bp32 = consts.tile([1, DIM], F32)
            nc.vector.tensor_copy(bp32[:], bp_sb[:])
            bpb = consts.tile([128, DIM], F32)
            nc.gpsimd.partition_broadcast(bpb[:], bp32[:], channels=128)

            qk_sb = qk_pool.tile([128, 12, N], F16)         # [o=1536, n]
            score_sb = score_pool.tile([128, CCH, N], F16)  # [c=768, n]
            v_sb = v_pool.tile([128, 8, N_HEAD * 65], F16)  # [n, h*(64+1)]

            # ---- Phase C: v projection, natural layout + ones cols ----
            v_ones = v_sb[:].rearrange("p n (h d) -> p n h d", d=65)[:, :, :, 64:65]
            nc.vector.memset(v_ones, 1.0)

            def c_group(nt, blk):
                o0, ow, off = ((0, 512, 0), (512, 256, 8 * 65))[blk]
                ps = ps_pool.tile([128, 512], F32)
                for c in range(CCH):
                    nc.tensor.matmul(
                        ps[:, :ow],
                        x_sb[:, c, nt * 128:(nt + 1) * 128],
                        wv_sb[:, c, o0:o0 + ow],
                        start=(c == 0),
                        stop=(c == CCH - 1),
                    )
                nh = ow // 64
                src = ps[:, :ow].rearrange("p (h d) -> p h d", d=64)
                bias = bvb[:, o0:o0 + ow].rearrange("p (h d) -> p h d", d=64)
                dst = v_sb[:, nt, off:off + nh * 65].rearrange(
                    "p (h d) -> p h d", d=65
                )[:, :, 0:64]
                nc.vector.tensor_add(dst, src, bias)

            # nt 0..1 upfront; nt 2..7 become wave-0 fillers so the score/exp
            # pipeline starts as early as possible (all C groups still finish
            # inside wave 0, before wave 1's PV needs v_sb)
            for nt in range(2):
                for blk in (0, 1):
                    c_group(nt, blk)

            # ---- Phase B helper: one [o-tile, nq] strip of the qk^T proj ----
            def b_group(ot, nq):
                ps = ps_pool.tile([128, 512], F32)
                for c in range(CCH):
                    nc.tensor.matmul(
                        ps[:],
                        wqk_sb[:, c, ot * 128:(ot + 1) * 128],
                        x_sb[:, c, nq:nq + 512],
                        start=(c == 0),
                        stop=(c == CCH - 1),
                    )
                nc.vector.tensor_scalar_add(
                    qk_sb[:, ot, nq:nq + 512], ps[:], bqk_sb[:, ot:ot + 1],
                )

            # ---- Phase D helpers ----
            def score_pair(hp, nq, nk):
                """scoresT for both heads of pair hp, one nk tile: head A into
                cols 0:512 (PE rows 0-63), head B into 512:1024 (rows 64-127),
                then exp straight from the 2-bank PSUM pair into fp16 SBUF."""
                pair = pair_pool.tile([128, 1024], F32)
                for half, p0 in ((0, 0), (1, 64)):
                    nc.tensor.matmul(
                        pair[:, half * 512:(half + 1) * 512],
                        qk_sb[p0:p0 + 64, 6 + hp, nk * 128:(nk + 1) * 128],
                        qk_sb[p0:p0 + 64, hp, nq:nq + 512],
                        start=True, stop=True,
                        tile_position=(p0, 0),
                    )
                at = attn_pool.tile([128, 1024], F16)
                nc.scalar.activation(at[:], pair[:], AF.Exp)
                return at

            def pv_group(hp, nq, half, p0, attns):
                """attn @ [v|1] for one head/nq strip + normalize by rowsum."""
                h = 2 * hp + half
                acc = acc_pool.tile([65, 512], F32)
                for nk in range(8):
                    nc.tensor.matmul(
                        acc[:],
                        v_sb[:, nk, h * 65:(h + 1) * 65],
                        attns[nk][:, half * 512:(half + 1) * 512],
                        start=(nk == 0),
                        stop=(nk == 7),
                    )
                # custom-DVE ops mis-read PSUM APs at partition offsets > 0 —
                # stage the rowsum row to SBUF first.
                rs = small_pool.tile([1, 512], F32, tag="rs")
                nc.vector.tensor_copy(rs[:], acc[64:65, :])
                rec = small_pool.tile([1, 512], F32, tag="rec")
                nc.vector.reciprocal_approx_fast(rec[:], rs[:])
                bc = small_pool.tile([64, 512], F32, tag="bc")
                nc.gpsimd.partition_broadcast(bc[:], rec[:], channels=64)
                nc.vector.tensor_mul(
                    score_sb[p0:p0 + 64, hp, nq:nq + 512], acc[0:64, :], bc[:],
                )

            # ---- Phases B + D interleaved in waves over head pairs ----
            b_group(0, 0)
            b_group(0, 512)
            b_group(6, 0)
            b_group(6, 512)
            prev_strips = None
            for hp in range(5):
                # fillers keep the PE busy while ScalarE exps this wave
                fillers = []
                if hp == 0:
                    for nt in (2, 3, 4, 5, 6, 7):
                        for blk in (0, 1):
                            fillers.append(
                                lambda nt=nt, blk=blk: c_group(nt, blk))
                if hp < 5:
                    for ot in (hp + 1, 7 + hp):
                        for nq in (0, 512):
                            fillers.append(lambda ot=ot, nq=nq: b_group(ot, nq))
                if prev_strips is not None:
                    php, pstrips = prev_strips
                    for nq in (0, 512):
                        for half, p0 in ((0, 0), (1, 64)):
                            fillers.append(
                                lambda nq=nq, half=half, p0=p0, php=php,
                                       s=pstrips: pv_group(php, nq, half, p0, s[nq])
                            )
                strips = {0: [], 512: []}
                fi = 0
                for si, (nq, nk) in enumerate(
                    [(nq, nk) for nq in (0, 512) for nk in range(8)]
                ):
                    strips[nq].append(score_pair(hp, nq, nk))
                    if si % 2 == 1 and fi < len(fillers):
                        fillers[fi]()
                        fi += 1
                while fi < len(fillers):
                    fillers[fi]()
                    fi += 1
                prev_strips = (hp, strips)

            # ---- Phase E helper ----
            def e_tile(nt):
                stage = out_pool.tile([128, DIM], F32)
                for o0, ow in ((0, 512), (512, 256)):
                    ps = ps_pool.tile([128, 512], F32)
                    for c in range(CCH):
                        nc.tensor.matmul(
                            ps[:, :ow],
                            score_sb[:, c, nt * 128:(nt + 1) * 128],
                            wp_sb[:, c, o0:o0 + ow],
                            start=(c == 0),
                            stop=(c == CCH - 1),
                        )
                    nc.vector.tensor_add(
                        stage[:, o0:o0 + ow], ps[:, :ow], bpb[:, o0:o0 + ow],
                    )
                nc.sync.dma_start(out_d[nt * 128:(nt + 1) * 128, :], stage[:])

            # ---- wave 5: scores for hp=5; pv(4) fills the nq0 half, then
            # pv(5,nq0) fills the nq1 half; the tail interleaves pv(5,nq1)
            # with E tiles (whose nq halves of score_sb are already final) ----
            _, p4 = prev_strips
            strips5 = {0: [], 512: []}
            for nk in range(8):
                strips5[0].append(score_pair(5, 0, nk))
                if nk % 2 == 1:
                    half, p0 = ((0, 0), (1, 64))[(nk // 2) % 2]
                    nq4 = 0 if nk < 4 else 512
                    pv_group(4, nq4, half, p0, p4[nq4])
            for nk in range(8):
                strips5[512].append(score_pair(5, 512, nk))
                if nk == 3:
                    pv_group(5, 0, 0, 0, strips5[0])
                elif nk == 7:
                    pv_group(5, 0, 1, 64, strips5[0])
            e_tile(0)
            e_tile(1)
            e_tile(2)
            pv_group(5, 512, 0, 0, strips5[512])
            e_tile(3)
            pv_group(5, 512, 1, 64, strips5[512])
            for nt in (4, 5, 6, 7):
                e_tile(nt)

    nc.compile()
    return nc


def _get_nc():
    if "nc" not in _CACHE:
        _CACHE["nc"] = _build()
    return _CACHE["nc"]


def kernel(x, w_qkv, b_qkv, w_proj, b_proj, **run_kwargs):
    x = np.asarray(x, dtype=np.float32)
    w_qkv = np.asarray(w_qkv, dtype=np.float32)
    b_qkv = np.asarray(b_qkv, dtype=np.float32)
    w_proj = np.asarray(w_proj, dtype=np.float32)
    b_proj = np.asarray(b_proj, dtype=np.float32)

    # Host-side layout prep (no arithmetic beyond folding the 1/sqrt(d) scale
    # into the q projection).
    w_qk = w_qkv[: 2 * DIM].copy()
    b_qk = b_qkv[: 2 * DIM].copy()
    w_qk[:DIM] *= SCALE
    b_qk[:DIM] *= SCALE
    w_qk_T = np.ascontiguousarray(w_qk.T).astype(np.float16)      # [768, 1536]
    b_qk_t = np.ascontiguousarray(b_qk.reshape(12, 128).T)        # [128, 12] f32
    w_v_T = np.ascontiguousarray(w_qkv[2 * DIM:].T).astype(np.float16)
    b_v = b_qkv[2 * DIM:].reshape(1, DIM).astype(np.float16)
    w_p_T = np.ascontiguousarray(w_proj.T).astype(np.float16)
    b_p = b_proj.reshape(1, DIM).astype(np.float16)

    nc = _get_nc()
    in_maps = []
    for b in range(NB):
        in_maps.append({
            "xT": np.ascontiguousarray(x[b].T).astype(np.float16),
            "w_qk_T": w_qk_T,
            "b_qk_t": b_qk_t,
            "w_v_T": w_v_T,
            "b_v": b_v,
            "w_p_T": w_p_T,
            "b_p": b_p,
        })
    res = run_bass_kernel_spmd(nc, in_maps, core_ids=list(range(NB)), **run_kwargs)
    out = np.stack([res.results[b]["out"] for b in range(NB)], axis=0)
    if run_kwargs:
        return out, res
    return out


if __name__ == "__main__":
    rng = np.random.default_rng(0)
    x = rng.standard_normal((NB, N, DIM), dtype=np.float32)
    w_qkv = rng.standard_normal((3 * DIM, DIM), dtype=np.float32) * DIM ** -0.5
    b_qkv = rng.standard_normal((3 * DIM,), dtype=np.float32) * 0.02
    w_proj = rng.standard_normal((DIM, DIM), dtype=np.float32) * DIM ** -0.5
    b_proj = rng.standard_normal((DIM,), dtype=np.float32) * 0.02
    out = kernel(x=x, w_qkv=w_qkv, b_qkv=b_qkv, w_proj=w_proj, b_proj=b_proj)
    print("out", out.shape, out.dtype, float(np.abs(out).mean()))
